# revision 1
# baseline (speedup 1.0000x reference)
"""Multi-head attention (B=4, S=2048, D=2048, H=16) on 8 trn2 NeuronCores.

Sharding: tensor-parallel over heads — 2 heads per core. Each core computes
its heads' Q/K/V projections, full attention for those heads, and a partial
output projection (its 256 rows of wo). The host sums the 8 partial outputs.

On-core layout: everything is kept "feature-major" ([d, token]) so that the
contraction dim always lands on SBUF partitions:
  - host ships xT [D, B*S] (tf32-pre-rounded, fp32r matmuls run at full PE rate)
  - QT/KT [128, tokens] per head come straight out of the projection matmuls
  - scores are computed transposed ([k, q]); exp is fused over two key-chunks
    per ACT instruction; the softmax denominator is a DVE tile-sum + one
    ones-matmul partition reduce; normalization is folded into the PSUM->SBUF
    copy of the unnormalized attention output (flash-style, no max needed
    since scores are ~N(0,1) in fp32).
  - the partial out-projection is interleaved per query-span so the PE has
    dense work while ACT works through the exps.
"""
import os
import sys

sys.path.insert(0, "/opt/trn_rl_repo")
import numpy as np

B, S, D, H = 4, 2048, 2048, 16
HD = 128
NCORES = 8
HP = H // NCORES          # heads per core = 2
DC = HP * HD              # per-core slice of D = 256
TOK = B * S               # 8192
SCALE = HD ** -0.5
NDC = D // 128            # 16 contraction chunks for the projections
SPAN = 256                # token span per projection step
NSPAN = S // SPAN         # 8 spans per batch
QS = 512                  # query span in attention
NQS = S // QS             # 4
NKC = S // 128            # 16 key chunks

LAST_EXEC_NS = None
_BUILT = None


def _round_tf32(x: np.ndarray) -> np.ndarray:
    """Round fp32 to tf32 (10 mantissa bits, RNE), keep fp32 container."""
    u = np.ascontiguousarray(x, dtype=np.float32).view(np.uint32)
    bias = np.uint32(0x00000FFF) + ((u >> np.uint32(13)) & np.uint32(1))
    return ((u + bias) & np.uint32(0xFFFFE000)).view(np.float32)


def _build():
    global _BUILT
    if _BUILT is not None:
        return _BUILT
    import concourse.tile as tile
    from concourse import bacc, mybir

    F32R = mybir.dt.float32r
    F32 = mybir.dt.float32
    Exp = mybir.ActivationFunctionType.Exp
    Ident = mybir.ActivationFunctionType.Identity

    nc = bacc.Bacc("TRN2", target_bir_lowering=False, debug=False)
    xt = nc.dram_tensor("xt", [D, TOK], F32R, kind="ExternalInput")
    wq = nc.dram_tensor("wq", [D, DC], F32R, kind="ExternalInput")
    wk = nc.dram_tensor("wk", [D, DC], F32R, kind="ExternalInput")
    wv = nc.dram_tensor("wv", [D, DC], F32R, kind="ExternalInput")
    wo = nc.dram_tensor("wo", [DC, D], F32R, kind="ExternalInput")
    bq2 = nc.dram_tensor("bq2", [HD, HP], F32, kind="ExternalInput")
    bk2 = nc.dram_tensor("bk2", [HD, HP], F32, kind="ExternalInput")
    ones = nc.dram_tensor("ones", [128, 128], F32R, kind="ExternalInput")
    out = nc.dram_tensor("out", [TOK, D], F32, kind="ExternalOutput")

    with tile.TileContext(nc) as tc:
        with tc.tile_pool(name="const", bufs=1) as cpool, \
             tc.tile_pool(name="xp", bufs=3) as xpool, \
             tc.tile_pool(name="bt", bufs=1) as bpool, \
             tc.tile_pool(name="at", bufs=3) as apool, \
             tc.tile_pool(name="ot", bufs=2) as opool, \
             tc.tile_pool(name="ps", bufs=1, space="PSUM") as ps:

            wq_sb = cpool.tile([128, NDC, DC], F32R)
            wk_sb = cpool.tile([128, NDC, DC], F32R)
            wv_sb = cpool.tile([128, NDC, DC], F32R)
            wo_sb = cpool.tile([128, HP, D], F32R)
            ones_sb = cpool.tile([128, 128], F32R)
            bq_sb = cpool.tile([HD, HP], F32)
            bk_sb = cpool.tile([HD, HP], F32)
            nc.sync.dma_start(out=wq_sb, in_=wq.rearrange("(c p) n -> p c n", p=128))
            nc.sync.dma_start(out=bq_sb, in_=bq2[:, :])
            nc.sync.dma_start(out=bk_sb, in_=bk2[:, :])

            xt_r = xt.rearrange("(c p) t -> p c t", p=128)

            for b in range(B):
                # ---- A) Q/K/V projections for batch b ----
                qt_b = bpool.tile([128, HP, S], F32R, name="qt_b", tag="qt_b")
                kt_b = bpool.tile([128, HP, S], F32R, name="kt_b", tag="kt_b")
                v_b = bpool.tile([128, NKC, DC], F32R, name="v_b", tag="v_b")
                for sp in range(NSPAN):
                    t0 = b * S + sp * SPAN
                    xsp = xpool.tile([128, NDC, SPAN], F32R, name="xsp", tag="xsp")
                    nc.sync.dma_start(out=xsp, in_=xt_r[:, :, t0:t0 + SPAN])
                    if b == 0 and sp == 0:
                        # wk/wv queue behind wq + the first x span so the PE
                        # can start the Q projection as early as possible
                        nc.sync.dma_start(
                            out=wk_sb, in_=wk.rearrange("(c p) n -> p c n", p=128))
                        nc.sync.dma_start(
                            out=wv_sb, in_=wv.rearrange("(c p) n -> p c n", p=128))
                    for h in range(HP):
                        # Q and K accumulate into halves of one PSUM bank
                        qkps = ps.tile([128, 2 * SPAN], F32, name="qkps",
                                       tag="pj", bufs=2)
                        for c in range(NDC):
                            nc.tensor.matmul(
                                qkps[:, 0:SPAN], wq_sb[:, c, h * HD:(h + 1) * HD],
                                xsp[:, c, :], start=(c == 0), stop=(c == NDC - 1))
                        for c in range(NDC):
                            nc.tensor.matmul(
                                qkps[:, SPAN:2 * SPAN],
                                wk_sb[:, c, h * HD:(h + 1) * HD],
                                xsp[:, c, :], start=(c == 0), stop=(c == NDC - 1))
                        nc.scalar.activation(
                            qt_b[:, h, sp * SPAN:(sp + 1) * SPAN],
                            qkps[:, 0:SPAN], Ident, bias=bq_sb[:, h:h + 1])
                        nc.scalar.activation(
                            kt_b[:, h, sp * SPAN:(sp + 1) * SPAN],
                            qkps[:, SPAN:2 * SPAN], Ident,
                            bias=bk_sb[:, h:h + 1])
                    # both V token-chunks accumulate into one PSUM bank
                    vps = ps.tile([128, 2 * DC], F32, name="vps", tag="pj",
                                  bufs=2)
                    for tch in range(SPAN // 128):
                        for c in range(NDC):
                            nc.tensor.matmul(
                                vps[:, tch * DC:(tch + 1) * DC],
                                xsp[:, c, tch * 128:(tch + 1) * 128],
                                wv_sb[:, c, :], start=(c == 0), stop=(c == NDC - 1))
                    for tch in range(SPAN // 128):
                        nc.scalar.copy(
                            v_b[:, sp * (SPAN // 128) + tch, :],
                            vps[:, tch * DC:(tch + 1) * DC])

                if b == 0:
                    # deferred so batch-0 x spans win the DMA queue at startup
                    nc.sync.dma_start(
                        out=wo_sb, in_=wo.rearrange("(c p) n -> p c n", p=128))
                    nc.sync.dma_start(out=ones_sb, in_=ones[:, :])

                # ---- B) attention + interleaved partial out-projection ----
                avt_b = bpool.tile([128, HP, S], F32R, name="avt_b", tag="avt_b")
                for qs in range(NQS):
                    for h in range(HP):
                        q_sl = qt_b[:, h, qs * QS:(qs + 1) * QS]
                        av_ps = ps.tile([HD, QS], F32, name="av_ps",
                                        tag="acc", bufs=2)
                        dn_ps = ps.tile([128, QS], F32, name="dn_ps",
                                        tag="acc", bufs=2)

                        def emit_av(kp, p_prev):
                            # AV and the softmax-denominator ones-matmul both
                            # consume the exp tile on the PE — keeps the PE
                            # dense (no DVE/GPSIMD reduction chains, no HAM
                            # cool-down gaps)
                            for j in range(2):
                                kc = 2 * kp + j
                                nc.tensor.matmul(
                                    av_ps, v_b[:, kc, h * HD:(h + 1) * HD],
                                    p_prev[:, j * QS:(j + 1) * QS],
                                    start=(kc == 0), stop=(kc == NKC - 1))
                            for j in range(2):
                                kc = 2 * kp + j
                                nc.tensor.matmul(
                                    dn_ps, ones_sb,
                                    p_prev[:, j * QS:(j + 1) * QS],
                                    start=(kc == 0), stop=(kc == NKC - 1))

                        p_prev = None
                        for kp in range(NKC // 2):
                            # two key-chunks share one psum tile and one exp;
                            # AV of pair kp-1 is emitted after the scores of
                            # pair kp so the PE never heads-of-line blocks on
                            # the exp it needs
                            s_ps = ps.tile([128, 2 * QS], F32, name="s_ps",
                                           tag="s", bufs=2)
                            p_sb = apool.tile([128, 2 * QS], F32R, name="p_sb",
                                              tag="p", bufs=3)
                            for j in range(2):
                                kc = 2 * kp + j
                                nc.tensor.matmul(
                                    s_ps[:, j * QS:(j + 1) * QS],
                                    kt_b[:, h, kc * 128:(kc + 1) * 128], q_sl,
                                    start=True, stop=True)
                            nc.scalar.activation(p_sb, s_ps, Exp, scale=SCALE)
                            if p_prev is not None:
                                emit_av(kp - 1, p_prev)
                            p_prev = p_sb
                        emit_av(NKC // 2 - 1, p_prev)
                        recip = apool.tile([128, QS], F32, name="recip",
                                           tag="recip", bufs=1)
                        nc.vector.reciprocal_approx_fast(recip, dn_ps)
                        nc.vector.tensor_mul(
                            avt_b[:, h, qs * QS:(qs + 1) * QS], av_ps, recip)

                    # partial out-projection for this query span (both heads
                    # are now done for tokens qs*QS .. (qs+1)*QS)
                    for tloc in range(QS // 128):
                        tch = qs * (QS // 128) + tloc
                        out_sb = opool.tile([128, D], F32, name="out_sb",
                                            tag="out_sb")
                        for dsp in range(D // 512):
                            ops = ps.tile([128, 512], F32, name="ops", tag="pj",
                                          bufs=2)
                            for h in range(HP):
                                nc.tensor.matmul(
                                    ops, avt_b[:, h, tch * 128:(tch + 1) * 128],
                                    wo_sb[:, h, dsp * 512:(dsp + 1) * 512],
                                    start=(h == 0), stop=(h == HP - 1))
                            nc.vector.tensor_copy(
                                out_sb[:, dsp * 512:(dsp + 1) * 512], ops)
                        nc.sync.dma_start(
                            out=out[b * S + tch * 128:b * S + (tch + 1) * 128, :],
                            in_=out_sb)
    nc.compile()
    _BUILT = nc
    return nc


def _install_trace_hooks():
    import types
    try:
        import antenv.axon_hooks  # noqa: F401
        return True
    except ImportError:
        pass
    try:
        from trn_agent_boot.trn_boot import _ntff_profile_via_ctypes
        hook = _ntff_profile_via_ctypes('/opt/axon/libaxon_pjrt.so')
        if hook is None:
            return False
        m = types.ModuleType('antenv.axon_hooks')
        m.get_axon_ntff_profile_hook = lambda: hook
        sys.modules['antenv.axon_hooks'] = m
        from concourse import bass_utils
        bass_utils.upload_artifacts = lambda tmpdir: "local://" + tmpdir
        return True
    except Exception:
        return False


def kernel(x, wq, bq, wk, bk, wv, bv, wo, bo):
    global LAST_EXEC_NS
    from concourse.bass_utils import run_bass_kernel_spmd

    x = np.asarray(x, dtype=np.float32)
    wq = np.asarray(wq, dtype=np.float32)
    bq = np.asarray(bq, dtype=np.float32)
    wk = np.asarray(wk, dtype=np.float32)
    bk = np.asarray(bk, dtype=np.float32)
    wv = np.asarray(wv, dtype=np.float32)
    bv = np.asarray(bv, dtype=np.float32)
    wo = np.asarray(wo, dtype=np.float32)
    bo = np.asarray(bo, dtype=np.float32)

    xt = _round_tf32(x.reshape(TOK, D).T)
    ones = np.ones((128, 128), dtype=np.float32)
    in_maps = []
    for i in range(NCORES):
        sl = slice(i * DC, (i + 1) * DC)
        in_maps.append({
            "xt": xt,
            "wq": _round_tf32(wq[:, sl]),
            "wk": _round_tf32(wk[:, sl]),
            "wv": _round_tf32(wv[:, sl]),
            "wo": _round_tf32(wo[sl, :]),
            "bq2": np.ascontiguousarray(bq[sl].reshape(HP, HD).T),
            "bk2": np.ascontiguousarray(bk[sl].reshape(HP, HD).T),
            "ones": ones,
        })

    trace = bool(os.environ.get("KERNEL_TRACE"))
    if trace:
        trace = _install_trace_hooks()

    nc = _build()
    res = run_bass_kernel_spmd(nc, in_maps, list(range(NCORES)), trace=trace)
    LAST_EXEC_NS = res.exec_time_ns

    total = np.zeros((TOK, D), dtype=np.float32)
    for r in res.results:
        total += r["out"]
    # V-bias folds into a constant row: softmax rows sum to 1, so
    # attention(V + 1*bv^T) = attention(V) + 1*bv^T, and (bv @ wo) adds to bo.
    total += bo + bv @ wo
    return total.reshape(B, S, D)



# revision 4
# speedup vs baseline: 1.0353x; 1.0353x over previous
"""Multi-head attention (B=4, S=2048, D=2048, H=16) on 8 trn2 NeuronCores.

Sharding: tensor-parallel over heads — 2 heads per core. Each core computes
its heads' Q/K/V projections, full attention for those heads, and a partial
output projection (its 256 rows of wo). The host sums the 8 partial outputs.

v2 (fp16 datapath, gap-free PE schedule):
  - every matmul operand is fp16 (x, weights, Q/K/V, exp tiles, attention
    output); PSUM accumulation stays fp32. fp16 keeps 10 mantissa bits
    (same relative precision as tf32) while halving DMA/SBUF/LDWEIGHTS.
  - softmax denominator: exp tiles are partial-summed on the DVE (fp16
    tensor_tensor, depth-2 tree -> two [128, 2*QS] accumulators per head/
    span), then 4 ones-matmuls partition-reduce into PSUM. Removes the
    512 N=512 denominator matmuls (~119us of PE) for ~8us of DVE per span.
  - the ACT exp stream (1106ns per pair-tile) is slightly slower than the
    scores+AV matmuls (864ns), so the PE would drift into exp-waits. The
    out-projection of span qs-1 is therefore emitted as 16 small groups
    interleaved INTO span qs's pair loops (and the last span's into the
    next batch's Q projection pass) — the PE always has dependency-free
    work queued and batch boundaries have no drain/pstate-ramp gap.
  - PSUM->SBUF output copies are split 4/12 between ACT and DVE to keep
    both helper engines below the PE rate.
  - b=0 startup: wq chunk-DMAs interleave with the first x span, then all
    x spans stream before wk/wv (Q pass is DMA-paced, K/V passes are not);
    first matmul at ~13us.
"""
import os
import sys

sys.path.insert(0, "/opt/trn_rl_repo")
import numpy as np

B, S, D, H = 4, 2048, 2048, 16
HD = 128
NCORES = 8
HP = H // NCORES          # heads per core = 2
DC = HP * HD              # per-core slice of D = 256
TOK = B * S               # 8192
SCALE = HD ** -0.5
NDC = D // 128            # 16 contraction chunks for the projections
SPAN = 256                # token span per projection step
NSPAN = S // SPAN         # 8 spans per batch
QS = 512                  # query span in attention
NQS = S // QS             # 4
NKC = S // 128            # 16 key chunks
NPAIR = NKC // 2          # 8 key-chunk pairs

LAST_EXEC_NS = None
_BUILT = None


def _build():
    global _BUILT
    if _BUILT is not None:
        return _BUILT
    import concourse.tile as tile
    from concourse import bacc, mybir

    F16 = mybir.dt.float16
    F32 = mybir.dt.float32
    Exp = mybir.ActivationFunctionType.Exp
    Ident = mybir.ActivationFunctionType.Identity

    nc = bacc.Bacc("TRN2", target_bir_lowering=False, debug=False)
    xt = nc.dram_tensor("xt", [D, TOK], F16, kind="ExternalInput")
    wq = nc.dram_tensor("wq", [D, DC], F16, kind="ExternalInput")
    wk = nc.dram_tensor("wk", [D, DC], F16, kind="ExternalInput")
    wv = nc.dram_tensor("wv", [D, DC], F16, kind="ExternalInput")
    wo = nc.dram_tensor("wo", [DC, D], F16, kind="ExternalInput")
    bq2 = nc.dram_tensor("bq2", [HD, HP], F32, kind="ExternalInput")
    bk2 = nc.dram_tensor("bk2", [HD, HP], F32, kind="ExternalInput")
    ones = nc.dram_tensor("ones", [128, 128], F16, kind="ExternalInput")
    out = nc.dram_tensor("out", [TOK, D], F32, kind="ExternalOutput")

    with tile.TileContext(nc) as tc:
        with tc.tile_pool(name="const", bufs=1) as cpool, \
             tc.tile_pool(name="xp", bufs=8) as xpool, \
             tc.tile_pool(name="bt", bufs=1) as bpool, \
             tc.tile_pool(name="qk", bufs=2) as qkpool, \
             tc.tile_pool(name="avp", bufs=2) as avpool, \
             tc.tile_pool(name="pp", bufs=4) as ppool, \
             tc.tile_pool(name="s2", bufs=4) as s2pool, \
             tc.tile_pool(name="rc", bufs=2) as rpool, \
             tc.tile_pool(name="ot", bufs=3) as opool, \
             tc.tile_pool(name="ps", bufs=1, space="PSUM") as ps:

            wq_sb = cpool.tile([128, NDC, DC], F16)
            wk_sb = cpool.tile([128, NDC, DC], F16)
            wv_sb = cpool.tile([128, NDC, DC], F16)
            wo_sb = cpool.tile([128, HP, D], F16)
            ones_sb = cpool.tile([128, 128], F16)
            bq_sb = cpool.tile([HD, HP], F32)
            bk_sb = cpool.tile([HD, HP], F32)

            wq_r = wq.rearrange("(c p) n -> p c n", p=128)
            wk_r = wk.rearrange("(c p) n -> p c n", p=128)
            wv_r = wv.rearrange("(c p) n -> p c n", p=128)
            wo_r = wo.rearrange("(c p) n -> p c n", p=128)
            xt_r = xt.rearrange("(c p) t -> p c t", p=128)

            nc.sync.dma_start(out=bq_sb, in_=bq2[:, :])
            nc.sync.dma_start(out=bk_sb, in_=bk2[:, :])

            xts = {}

            def x_dma(b, sp):
                t0 = b * S + sp * SPAN
                xtl = xpool.tile([128, NDC, SPAN], F16, name=f"x{b}_{sp}",
                                 tag="x")
                xts[(b, sp)] = xtl
                nc.sync.dma_start(out=xtl, in_=xt_r[:, :, t0:t0 + SPAN])

            # --- b=0 startup: wq chunks interleave with the first x span;
            # remaining x spans stream before wk/wv (the Q pass is DMA-paced,
            # the K/V passes run much later) ---
            x00 = xpool.tile([128, NDC, SPAN], F16, name="x0_0", tag="x")
            xts[(0, 0)] = x00
            for i in range(4):
                nc.sync.dma_start(out=wq_sb[:, 4 * i:4 * i + 4, :],
                                  in_=wq_r[:, 4 * i:4 * i + 4, :])
                nc.sync.dma_start(out=x00[:, 4 * i:4 * i + 4, :],
                                  in_=xt_r[:, 4 * i:4 * i + 4, 0:SPAN])
            for sp in range(1, NSPAN):
                x_dma(0, sp)
            for i in range(2):
                nc.sync.dma_start(out=wk_sb[:, 8 * i:8 * i + 8, :],
                                  in_=wk_r[:, 8 * i:8 * i + 8, :])
            for i in range(2):
                nc.sync.dma_start(out=wv_sb[:, 8 * i:8 * i + 8, :],
                                  in_=wv_r[:, 8 * i:8 * i + 8, :])

            def pull(filler):
                if filler is not None:
                    next(filler, None)

            def proj_pass(b, w_sb, b_sb, dst, filler=None):
                for sp in range(NSPAN):
                    xtl = xts[(b, sp)]
                    for h in range(HP):
                        pps = ps.tile([128, SPAN], F32, name="pps", tag="pj",
                                      bufs=2)
                        for c in range(NDC):
                            nc.tensor.matmul(
                                pps, w_sb[:, c, h * HD:(h + 1) * HD],
                                xtl[:, c, :], start=(c == 0),
                                stop=(c == NDC - 1))
                        nc.scalar.activation(
                            dst[:, h, sp * SPAN:(sp + 1) * SPAN], pps, Ident,
                            bias=b_sb[:, h:h + 1])
                        pull(filler)

            def v_pass(b, v_b):
                for sp in range(NSPAN):
                    xtl = xts[(b, sp)]
                    vps = ps.tile([128, 2 * DC], F32, name="vps", tag="pj",
                                  bufs=2)
                    for tch in range(2):
                        for c in range(NDC):
                            nc.tensor.matmul(
                                vps[:, tch * DC:(tch + 1) * DC],
                                xtl[:, c, tch * 128:(tch + 1) * 128],
                                wv_sb[:, c, :], start=(c == 0),
                                stop=(c == NDC - 1))
                    for tch in range(2):
                        nc.scalar.copy(v_b[:, sp * 2 + tch, :],
                                       vps[:, tch * DC:(tch + 1) * DC])

            def attn_span(qs, h, qt_b, kt_b, v_b, avt_b, filler=None):
                q_sl = qt_b[:, h, qs * QS:(qs + 1) * QS]
                av_ps = ps.tile([HD, QS], F32, name="av_ps", tag="acc",
                                bufs=2)
                sA = s2pool.tile([128, 2 * QS], F16, name="sA", tag="s2")
                sB = s2pool.tile([128, 2 * QS], F16, name="sB", tag="s2")
                p_tiles = []
                dn_ps = None

                def emit_av(kp):
                    pt = p_tiles[kp]
                    for j in range(2):
                        kc = 2 * kp + j
                        nc.tensor.matmul(
                            av_ps, v_b[:, kc, h * HD:(h + 1) * HD],
                            pt[:, j * QS:(j + 1) * QS], start=(kc == 0),
                            stop=(kc == NKC - 1))

                for kp in range(NPAIR):
                    s_ps = ps.tile([128, 2 * QS], F32, name="s_ps", tag="s",
                                   bufs=2)
                    for j in range(2):
                        kc = 2 * kp + j
                        nc.tensor.matmul(
                            s_ps[:, j * QS:(j + 1) * QS],
                            kt_b[:, h, kc * 128:(kc + 1) * 128], q_sl,
                            start=True, stop=True)
                    pt = ppool.tile([128, 2 * QS], F16, name="p_sb", tag="p")
                    nc.scalar.activation(pt, s_ps, Exp, scale=SCALE)
                    p_tiles.append(pt)
                    # DVE partial sums of the exp tiles (depth-2 tree)
                    if kp == 1:
                        nc.vector.tensor_add(sA, p_tiles[0], p_tiles[1])
                    elif kp in (2, 3):
                        nc.vector.tensor_add(sA, sA, p_tiles[kp])
                    elif kp == 5:
                        nc.vector.tensor_add(sB, p_tiles[4], p_tiles[5])
                    elif kp in (6, 7):
                        nc.vector.tensor_add(sB, sB, p_tiles[kp])
                    if kp >= 1:
                        emit_av(kp - 1)
                    if kp == 5:
                        # partition-reduce sA early (its adds are long done)
                        dn_ps = ps.tile([128, QS], F32, name="dn_ps",
                                        tag="acc", bufs=2)
                        nc.tensor.matmul(dn_ps, ones_sb, sA[:, 0:QS],
                                         start=True, stop=False)
                        nc.tensor.matmul(dn_ps, ones_sb, sA[:, QS:2 * QS],
                                         start=False, stop=False)
                    if kp >= 2:
                        pull(filler)
                emit_av(NPAIR - 1)
                nc.tensor.matmul(dn_ps, ones_sb, sB[:, 0:QS],
                                 start=False, stop=False)
                nc.tensor.matmul(dn_ps, ones_sb, sB[:, QS:2 * QS],
                                 start=False, stop=True)
                pull(filler)
                pull(filler)
                recip = rpool.tile([128, QS], F32, name="recip", tag="rc")
                nc.vector.reciprocal_approx_fast(recip, dn_ps)
                nc.vector.tensor_mul(
                    avt_b[:, h, qs * QS:(qs + 1) * QS], av_ps, recip)

            def outproj_gen(b, qs, avt_b, split):
                for tloc in range(QS // 128):
                    tch = qs * (QS // 128) + tloc
                    out_sb = opool.tile([128, D], F32, name="out_sb",
                                        tag="ot")
                    for dsp in range(4):
                        ops = ps.tile([128, 512], F32, name="ops", tag="pj",
                                      bufs=2)
                        for h in range(HP):
                            nc.tensor.matmul(
                                ops, avt_b[:, h, tch * 128:(tch + 1) * 128],
                                wo_sb[:, h, dsp * 512:(dsp + 1) * 512],
                                start=(h == 0), stop=(h == HP - 1))
                        if split[dsp] == "v":
                            nc.vector.tensor_copy(
                                out_sb[:, dsp * 512:(dsp + 1) * 512], ops)
                        else:
                            nc.scalar.copy(
                                out_sb[:, dsp * 512:(dsp + 1) * 512], ops)
                        if dsp == 3:
                            nc.sync.dma_start(
                                out=out[b * S + tch * 128:
                                        b * S + (tch + 1) * 128, :],
                                in_=out_sb)
                        yield

            carry = None          # half-consumed outproj of (b-1, qs=3)
            for b in range(B):
                qt_b = qkpool.tile([128, HP, S], F16, name="qt_b", tag="qt")
                kt_b = qkpool.tile([128, HP, S], F16, name="kt_b", tag="kt")
                v_b = bpool.tile([128, NKC, DC], F16, name="v_b", tag="v")
                avt_b = avpool.tile([128, HP, S], F16, name="avt_b",
                                    tag="avt")

                # previous batch's last-span out-projection rides the Q pass
                proj_pass(b, wq_sb, bq_sb, qt_b, filler=carry)
                proj_pass(b, wk_sb, bk_sb, kt_b)
                v_pass(b, v_b)

                if b == 0:
                    for i in range(4):
                        nc.sync.dma_start(
                            out=wo_sb[:, :, 512 * i:512 * (i + 1)],
                            in_=wo_r[:, :, 512 * i:512 * (i + 1)])
                    nc.sync.dma_start(out=ones_sb, in_=ones[:, :])

                for qs in range(NQS):
                    if qs == 0:
                        filler = carry       # leftovers (may be exhausted)
                    else:
                        filler = outproj_gen(b, qs - 1, avt_b, "svvv")
                    attn_span(qs, 0, qt_b, kt_b, v_b, avt_b, filler)
                    if qs == 0 and b + 1 < B:
                        for sp in range(NSPAN):
                            x_dma(b + 1, sp)
                    attn_span(qs, 1, qt_b, kt_b, v_b, avt_b, filler)
                    if filler is not None:
                        for _ in filler:     # drain any leftovers
                            pass
                carry = outproj_gen(b, NQS - 1, avt_b, "vvvv")

            if carry is not None:            # last batch's final span
                for _ in carry:
                    pass
    nc.compile()
    _BUILT = nc
    return nc


def _install_trace_hooks():
    import types
    try:
        import antenv.axon_hooks  # noqa: F401
        return True
    except ImportError:
        pass
    try:
        from trn_agent_boot.trn_boot import _ntff_profile_via_ctypes
        hook = _ntff_profile_via_ctypes('/opt/axon/libaxon_pjrt.so')
        if hook is None:
            return False
        m = types.ModuleType('antenv.axon_hooks')
        m.get_axon_ntff_profile_hook = lambda: hook
        sys.modules['antenv.axon_hooks'] = m
        from concourse import bass_utils
        bass_utils.upload_artifacts = lambda tmpdir: "local://" + tmpdir
        return True
    except Exception:
        return False


def kernel(x, wq, bq, wk, bk, wv, bv, wo, bo):
    global LAST_EXEC_NS
    from concourse.bass_utils import run_bass_kernel_spmd

    x = np.asarray(x, dtype=np.float32)
    wq = np.asarray(wq, dtype=np.float32)
    bq = np.asarray(bq, dtype=np.float32)
    wk = np.asarray(wk, dtype=np.float32)
    bk = np.asarray(bk, dtype=np.float32)
    wv = np.asarray(wv, dtype=np.float32)
    bv = np.asarray(bv, dtype=np.float32)
    wo = np.asarray(wo, dtype=np.float32)
    bo = np.asarray(bo, dtype=np.float32)

    xt = np.ascontiguousarray(x.reshape(TOK, D).T).astype(np.float16)
    ones = np.ones((128, 128), dtype=np.float16)
    in_maps = []
    for i in range(NCORES):
        sl = slice(i * DC, (i + 1) * DC)
        in_maps.append({
            "xt": xt,
            "wq": np.ascontiguousarray(wq[:, sl]).astype(np.float16),
            "wk": np.ascontiguousarray(wk[:, sl]).astype(np.float16),
            "wv": np.ascontiguousarray(wv[:, sl]).astype(np.float16),
            "wo": np.ascontiguousarray(wo[sl, :]).astype(np.float16),
            "bq2": np.ascontiguousarray(bq[sl].reshape(HP, HD).T),
            "bk2": np.ascontiguousarray(bk[sl].reshape(HP, HD).T),
            "ones": ones,
        })

    trace = bool(os.environ.get("KERNEL_TRACE"))
    if trace:
        trace = _install_trace_hooks()

    nc = _build()
    res = run_bass_kernel_spmd(nc, in_maps, list(range(NCORES)), trace=trace)
    LAST_EXEC_NS = res.exec_time_ns

    total = np.zeros((TOK, D), dtype=np.float32)
    for r in res.results:
        total += r["out"]
    # V-bias folds into a constant row: softmax rows sum to 1, so
    # attention(V + 1*bv^T) = attention(V) + 1*bv^T, and (bv @ wo) adds to bo.
    total += bo + bv @ wo
    return total.reshape(B, S, D)


# revision 5
# speedup vs baseline: 1.1893x; 1.1488x over previous
"""Multi-head attention (B=4, S=2048, D=2048, H=16) on 8 trn2 NeuronCores.

Sharding: tensor-parallel over heads — 2 heads per core. Each core computes
its heads' Q/K/V projections, full attention for those heads, and a partial
output projection (its 256 rows of wo). The host sums the 8 partial outputs.

v2 (fp16 datapath, gap-free PE schedule):
  - every matmul operand is fp16 (x, weights, Q/K/V, exp tiles, attention
    output); PSUM accumulation stays fp32. fp16 keeps 10 mantissa bits
    (same relative precision as tf32) while halving DMA/SBUF/LDWEIGHTS.
  - softmax denominator: exp tiles are partial-summed on the DVE (fp16
    tensor_tensor, depth-2 tree -> two [128, 2*QS] accumulators per head/
    span), then 4 ones-matmuls partition-reduce into PSUM. Removes the
    512 N=512 denominator matmuls (~119us of PE) for ~8us of DVE per span.
  - the ACT exp stream (1106ns per pair-tile) is slightly slower than the
    scores+AV matmuls (864ns), so the PE would drift into exp-waits. The
    out-projection of span qs-1 is therefore emitted as 16 small groups
    interleaved INTO span qs's pair loops (and the last span's into the
    next batch's Q projection pass) — the PE always has dependency-free
    work queued and batch boundaries have no drain/pstate-ramp gap.
  - PSUM->SBUF output copies are split 4/12 between ACT and DVE to keep
    both helper engines below the PE rate.
  - b=0 startup: wq chunk-DMAs interleave with the first x span, then all
    x spans stream before wk/wv (Q pass is DMA-paced, K/V passes are not);
    first matmul at ~13us.
"""
import os
import sys

sys.path.insert(0, "/opt/trn_rl_repo")
import numpy as np

B, S, D, H = 4, 2048, 2048, 16
HD = 128
NCORES = 8
HP = H // NCORES          # heads per core = 2
DC = HP * HD              # per-core slice of D = 256
TOK = B * S               # 8192
SCALE = HD ** -0.5
NDC = D // 128            # 16 contraction chunks for the projections
SPAN = 256                # token span per projection step
NSPAN = S // SPAN         # 8 spans per batch
QS = 512                  # query span in attention
NQS = S // QS             # 4
NKC = S // 128            # 16 key chunks
NPAIR = NKC // 2          # 8 key-chunk pairs

LAST_EXEC_NS = None
_BUILT = None


def _build():
    global _BUILT
    if _BUILT is not None:
        return _BUILT
    import concourse.tile as tile
    from concourse import bacc, mybir

    F16 = mybir.dt.float16
    F32 = mybir.dt.float32
    Exp = mybir.ActivationFunctionType.Exp
    Ident = mybir.ActivationFunctionType.Identity

    nc = bacc.Bacc("TRN2", target_bir_lowering=False, debug=False)
    xt = nc.dram_tensor("xt", [D, TOK], F16, kind="ExternalInput")
    wq = nc.dram_tensor("wq", [D, DC], F16, kind="ExternalInput")
    wk = nc.dram_tensor("wk", [D, DC], F16, kind="ExternalInput")
    wv = nc.dram_tensor("wv", [D, DC], F16, kind="ExternalInput")
    wo = nc.dram_tensor("wo", [DC, D], F16, kind="ExternalInput")
    bq2 = nc.dram_tensor("bq2", [HD, HP], F32, kind="ExternalInput")
    bk2 = nc.dram_tensor("bk2", [HD, HP], F32, kind="ExternalInput")
    ones = nc.dram_tensor("ones", [128, 128], F16, kind="ExternalInput")
    out = nc.dram_tensor("out", [TOK, D], F32, kind="ExternalOutput")

    with tile.TileContext(nc) as tc:
        with tc.tile_pool(name="const", bufs=1) as cpool, \
             tc.tile_pool(name="xp", bufs=8) as xpool, \
             tc.tile_pool(name="bt", bufs=1) as bpool, \
             tc.tile_pool(name="qk", bufs=2) as qkpool, \
             tc.tile_pool(name="avp", bufs=2) as avpool, \
             tc.tile_pool(name="pp", bufs=4) as ppool, \
             tc.tile_pool(name="s2", bufs=6) as s2pool, \
             tc.tile_pool(name="rc", bufs=2) as rpool, \
             tc.tile_pool(name="ot", bufs=3) as opool, \
             tc.tile_pool(name="ps", bufs=1, space="PSUM") as ps:

            wq_sb = cpool.tile([128, NDC, DC], F16)
            wk_sb = cpool.tile([128, NDC, DC], F16)
            wv_sb = cpool.tile([128, NDC, DC], F16)
            wo_sb = cpool.tile([128, HP, D], F16)
            ones_sb = cpool.tile([128, 128], F16)
            bq_sb = cpool.tile([HD, HP], F32)
            bk_sb = cpool.tile([HD, HP], F32)

            wq_r = wq.rearrange("(c p) n -> p c n", p=128)
            wk_r = wk.rearrange("(c p) n -> p c n", p=128)
            wv_r = wv.rearrange("(c p) n -> p c n", p=128)
            wo_r = wo.rearrange("(c p) n -> p c n", p=128)
            xt_r = xt.rearrange("(c p) t -> p c t", p=128)

            nc.sync.dma_start(out=bq_sb, in_=bq2[:, :])
            nc.sync.dma_start(out=bk_sb, in_=bk2[:, :])

            xts = {}

            def x_dma(b, sp):
                t0 = b * S + sp * SPAN
                xtl = xpool.tile([128, NDC, SPAN], F16, name=f"x{b}_{sp}",
                                 tag="x")
                xts[(b, sp)] = xtl
                nc.sync.dma_start(out=xtl, in_=xt_r[:, :, t0:t0 + SPAN])

            # --- b=0 startup: wq chunks interleave with the first x span;
            # remaining x spans stream before wk/wv (the Q pass is DMA-paced,
            # the K/V passes run much later) ---
            x00 = xpool.tile([128, NDC, SPAN], F16, name="x0_0", tag="x")
            xts[(0, 0)] = x00
            for i in range(4):
                nc.sync.dma_start(out=wq_sb[:, 4 * i:4 * i + 4, :],
                                  in_=wq_r[:, 4 * i:4 * i + 4, :])
                nc.sync.dma_start(out=x00[:, 4 * i:4 * i + 4, :],
                                  in_=xt_r[:, 4 * i:4 * i + 4, 0:SPAN])
            for sp in range(1, NSPAN):
                x_dma(0, sp)
            for i in range(2):
                nc.sync.dma_start(out=wk_sb[:, 8 * i:8 * i + 8, :],
                                  in_=wk_r[:, 8 * i:8 * i + 8, :])
            for i in range(2):
                nc.sync.dma_start(out=wv_sb[:, 8 * i:8 * i + 8, :],
                                  in_=wv_r[:, 8 * i:8 * i + 8, :])

            def pull(filler):
                if filler is not None:
                    next(filler, None)

            def proj_pass(b, w_sb, b_sb, dst):
                for sp in range(NSPAN):
                    xtl = xts[(b, sp)]
                    for h in range(HP):
                        pps = ps.tile([128, SPAN], F32, name="pps", tag="pj",
                                      bufs=2)
                        for c in range(NDC):
                            nc.tensor.matmul(
                                pps, w_sb[:, c, h * HD:(h + 1) * HD],
                                xtl[:, c, :], start=(c == 0),
                                stop=(c == NDC - 1))
                        nc.scalar.activation(
                            dst[:, h, sp * SPAN:(sp + 1) * SPAN], pps, Ident,
                            bias=b_sb[:, h:h + 1])

            def v_pass(b, v_b):
                for sp in range(NSPAN):
                    xtl = xts[(b, sp)]
                    vps = ps.tile([128, 2 * DC], F32, name="vps", tag="pj",
                                  bufs=2)
                    for tch in range(2):
                        for c in range(NDC):
                            nc.tensor.matmul(
                                vps[:, tch * DC:(tch + 1) * DC],
                                xtl[:, c, tch * 128:(tch + 1) * 128],
                                wv_sb[:, c, :], start=(c == 0),
                                stop=(c == NDC - 1))
                    for tch in range(2):
                        nc.scalar.copy(v_b[:, sp * 2 + tch, :],
                                       vps[:, tch * DC:(tch + 1) * DC])

            def attn_span(qs, h, qt_b, kt_b, v_b, avt_b, filler=None):
                q_sl = qt_b[:, h, qs * QS:(qs + 1) * QS]
                av_ps = ps.tile([HD, QS], F32, name="av_ps", tag="acc",
                                bufs=2)
                p_tiles = []
                t_tiles = []
                dn_ps = None

                def emit_av(kp):
                    pt = p_tiles[kp]
                    for j in range(2):
                        kc = 2 * kp + j
                        nc.tensor.matmul(
                            av_ps, v_b[:, kc, h * HD:(h + 1) * HD],
                            pt[:, j * QS:(j + 1) * QS], start=(kc == 0),
                            stop=(kc == NKC - 1))

                def dn_mm(src, first=False, last=False):
                    nc.tensor.matmul(dn_ps, ones_sb, src[:, 0:QS],
                                     start=first, stop=False)
                    nc.tensor.matmul(dn_ps, ones_sb, src[:, QS:2 * QS],
                                     start=False, stop=last)

                for kp in range(NPAIR):
                    s_ps = ps.tile([128, 2 * QS], F32, name="s_ps", tag="s",
                                   bufs=2)
                    for j in range(2):
                        kc = 2 * kp + j
                        nc.tensor.matmul(
                            s_ps[:, j * QS:(j + 1) * QS],
                            kt_b[:, h, kc * 128:(kc + 1) * 128], q_sl,
                            start=True, stop=True)
                    pt = ppool.tile([128, 2 * QS], F16, name="p_sb", tag="p")
                    nc.scalar.activation(pt, s_ps, Exp, scale=SCALE)
                    p_tiles.append(pt)
                    # DVE pair-sums for pairs 0-2; the last pair feeds the
                    # ones-matmuls directly (keeps the span tail short)
                    if kp in (1, 3, 5):
                        tt = s2pool.tile([128, 2 * QS], F16, name="t_sb",
                                         tag="s2")
                        nc.vector.tensor_add(tt, p_tiles[kp - 1],
                                             p_tiles[kp])
                        t_tiles.append(tt)
                    if kp >= 1:
                        emit_av(kp - 1)
                    if kp >= 5:
                        # partition-reduce the pair-sums (adds long done)
                        if kp == 5:
                            dn_ps = ps.tile([128, QS], F32, name="dn_ps",
                                            tag="acc", bufs=2)
                        dn_mm(t_tiles[kp - 5], first=(kp == 5))
                    if kp >= 2:
                        pull(filler)
                emit_av(NPAIR - 1)
                dn_mm(p_tiles[6])
                dn_mm(p_tiles[7], last=True)
                recip = rpool.tile([128, QS], F32, name="recip", tag="rc")
                nc.vector.reciprocal_approx_fast(recip, dn_ps)
                nc.vector.tensor_mul(
                    avt_b[:, h, qs * QS:(qs + 1) * QS], av_ps, recip)
                pull(filler)
                pull(filler)

            def outproj_gen(b, qs, avt_b, split):
                for tloc in range(QS // 128):
                    tch = qs * (QS // 128) + tloc
                    out_sb = opool.tile([128, D], F32, name="out_sb",
                                        tag="ot")
                    for dsp in range(4):
                        ops = ps.tile([128, 512], F32, name="ops", tag="pj",
                                      bufs=2)
                        for h in range(HP):
                            nc.tensor.matmul(
                                ops, avt_b[:, h, tch * 128:(tch + 1) * 128],
                                wo_sb[:, h, dsp * 512:(dsp + 1) * 512],
                                start=(h == 0), stop=(h == HP - 1))
                        if split[dsp] == "v":
                            nc.vector.tensor_copy(
                                out_sb[:, dsp * 512:(dsp + 1) * 512], ops)
                        else:
                            nc.scalar.copy(
                                out_sb[:, dsp * 512:(dsp + 1) * 512], ops)
                        if dsp == 3:
                            nc.sync.dma_start(
                                out=out[b * S + tch * 128:
                                        b * S + (tch + 1) * 128, :],
                                in_=out_sb)
                        yield

            carry = None          # half-consumed outproj of (b-1, qs=3)
            for b in range(B):
                qt_b = qkpool.tile([128, HP, S], F16, name="qt_b", tag="qt")
                kt_b = qkpool.tile([128, HP, S], F16, name="kt_b", tag="kt")
                v_b = bpool.tile([128, NKC, DC], F16, name="v_b", tag="v")
                avt_b = avpool.tile([128, HP, S], F16, name="avt_b",
                                    tag="avt")

                proj_pass(b, wq_sb, bq_sb, qt_b)
                proj_pass(b, wk_sb, bk_sb, kt_b)
                v_pass(b, v_b)

                if b == 0:
                    for i in range(4):
                        nc.sync.dma_start(
                            out=wo_sb[:, :, 512 * i:512 * (i + 1)],
                            in_=wo_r[:, :, 512 * i:512 * (i + 1)])
                    nc.sync.dma_start(out=ones_sb, in_=ones[:, :])

                for qs in range(NQS):
                    if qs == 0:
                        filler = carry       # leftovers (may be exhausted)
                    else:
                        filler = outproj_gen(b, qs - 1, avt_b, "vvvv")
                    attn_span(qs, 0, qt_b, kt_b, v_b, avt_b, filler)
                    if qs == 0 and b + 1 < B:
                        for sp in range(NSPAN):
                            x_dma(b + 1, sp)
                    attn_span(qs, 1, qt_b, kt_b, v_b, avt_b, filler)
                    if filler is not None:
                        for _ in filler:     # drain any leftovers
                            pass
                carry = outproj_gen(b, NQS - 1, avt_b,
                                    "vvvv" if b + 1 < B else "svsv")

            if carry is not None:            # last batch's final span:
                for _ in carry:                  # drain with copies split
                    pass                         # across ACT+DVE (both idle)
    nc.compile()
    _BUILT = nc
    return nc


def _install_trace_hooks():
    import types
    try:
        import antenv.axon_hooks  # noqa: F401
        return True
    except ImportError:
        pass
    try:
        from trn_agent_boot.trn_boot import _ntff_profile_via_ctypes
        hook = _ntff_profile_via_ctypes('/opt/axon/libaxon_pjrt.so')
        if hook is None:
            return False
        m = types.ModuleType('antenv.axon_hooks')
        m.get_axon_ntff_profile_hook = lambda: hook
        sys.modules['antenv.axon_hooks'] = m
        from concourse import bass_utils
        bass_utils.upload_artifacts = lambda tmpdir: "local://" + tmpdir
        return True
    except Exception:
        return False


def kernel(x, wq, bq, wk, bk, wv, bv, wo, bo):
    global LAST_EXEC_NS
    from concourse.bass_utils import run_bass_kernel_spmd

    x = np.asarray(x, dtype=np.float32)
    wq = np.asarray(wq, dtype=np.float32)
    bq = np.asarray(bq, dtype=np.float32)
    wk = np.asarray(wk, dtype=np.float32)
    bk = np.asarray(bk, dtype=np.float32)
    wv = np.asarray(wv, dtype=np.float32)
    bv = np.asarray(bv, dtype=np.float32)
    wo = np.asarray(wo, dtype=np.float32)
    bo = np.asarray(bo, dtype=np.float32)

    xt = np.ascontiguousarray(x.reshape(TOK, D).T).astype(np.float16)
    ones = np.ones((128, 128), dtype=np.float16)
    in_maps = []
    for i in range(NCORES):
        sl = slice(i * DC, (i + 1) * DC)
        in_maps.append({
            "xt": xt,
            "wq": np.ascontiguousarray(wq[:, sl]).astype(np.float16),
            "wk": np.ascontiguousarray(wk[:, sl]).astype(np.float16),
            "wv": np.ascontiguousarray(wv[:, sl]).astype(np.float16),
            "wo": np.ascontiguousarray(wo[sl, :]).astype(np.float16),
            "bq2": np.ascontiguousarray(bq[sl].reshape(HP, HD).T),
            "bk2": np.ascontiguousarray(bk[sl].reshape(HP, HD).T),
            "ones": ones,
        })

    trace = bool(os.environ.get("KERNEL_TRACE"))
    if trace:
        trace = _install_trace_hooks()

    nc = _build()
    res = run_bass_kernel_spmd(nc, in_maps, list(range(NCORES)), trace=trace)
    LAST_EXEC_NS = res.exec_time_ns

    total = np.zeros((TOK, D), dtype=np.float32)
    for r in res.results:
        total += r["out"]
    # V-bias folds into a constant row: softmax rows sum to 1, so
    # attention(V + 1*bv^T) = attention(V) + 1*bv^T, and (bv @ wo) adds to bo.
    total += bo + bv @ wo
    return total.reshape(B, S, D)


# revision 6
# speedup vs baseline: 1.2294x; 1.0337x over previous
"""Multi-head attention (B=4, S=2048, D=2048, H=16) on 8 trn2 NeuronCores.

Sharding: tensor-parallel over heads — 2 heads per core. Each core computes
its heads' Q/K/V projections, full attention for those heads, and a partial
output projection (its 256 rows of wo). The host sums the 8 partial outputs.

v2 (fp16 datapath, gap-free PE schedule):
  - every matmul operand is fp16 (x, weights, Q/K/V, exp tiles, attention
    output); PSUM accumulation stays fp32. fp16 keeps 10 mantissa bits
    (same relative precision as tf32) while halving DMA/SBUF/LDWEIGHTS.
  - softmax denominator: exp tiles are partial-summed on the DVE (fp16
    tensor_tensor, depth-2 tree -> two [128, 2*QS] accumulators per head/
    span), then 4 ones-matmuls partition-reduce into PSUM. Removes the
    512 N=512 denominator matmuls (~119us of PE) for ~8us of DVE per span.
  - the ACT exp stream (1106ns per pair-tile) is slightly slower than the
    scores+AV matmuls (864ns), so the PE would drift into exp-waits. The
    out-projection of span qs-1 is therefore emitted as 16 small groups
    interleaved INTO span qs's pair loops (and the last span's into the
    next batch's Q projection pass) — the PE always has dependency-free
    work queued and batch boundaries have no drain/pstate-ramp gap.
  - PSUM->SBUF output copies are split 4/12 between ACT and DVE to keep
    both helper engines below the PE rate.
  - b=0 startup: wq chunk-DMAs interleave with the first x span, then all
    x spans stream before wk/wv (Q pass is DMA-paced, K/V passes are not);
    first matmul at ~13us.
"""
import os
import sys

sys.path.insert(0, "/opt/trn_rl_repo")
import numpy as np

B, S, D, H = 4, 2048, 2048, 16
HD = 128
NCORES = 8
HP = H // NCORES          # heads per core = 2
DC = HP * HD              # per-core slice of D = 256
TOK = B * S               # 8192
SCALE = HD ** -0.5
NDC = D // 128            # 16 contraction chunks for the projections
SPAN = 256                # token span per projection step
NSPAN = S // SPAN         # 8 spans per batch
QS = 512                  # query span in attention
NQS = S // QS             # 4
NKC = S // 128            # 16 key chunks
NPAIR = NKC // 2          # 8 key-chunk pairs

LAST_EXEC_NS = None
_BUILT = None


def _build():
    global _BUILT
    if _BUILT is not None:
        return _BUILT
    import concourse.tile as tile
    from concourse import bacc, mybir

    F16 = mybir.dt.float16
    F32 = mybir.dt.float32
    Exp = mybir.ActivationFunctionType.Exp
    Ident = mybir.ActivationFunctionType.Identity

    nc = bacc.Bacc("TRN2", target_bir_lowering=False, debug=False)
    xt = nc.dram_tensor("xt", [D, TOK], F16, kind="ExternalInput")
    wq = nc.dram_tensor("wq", [D, DC], F16, kind="ExternalInput")
    wk = nc.dram_tensor("wk", [D, DC], F16, kind="ExternalInput")
    wv = nc.dram_tensor("wv", [D, DC], F16, kind="ExternalInput")
    wo = nc.dram_tensor("wo", [DC, D], F16, kind="ExternalInput")
    bq2 = nc.dram_tensor("bq2", [HD, HP], F32, kind="ExternalInput")
    bk2 = nc.dram_tensor("bk2", [HD, HP], F32, kind="ExternalInput")
    ones = nc.dram_tensor("ones", [128, 128], F16, kind="ExternalInput")
    out = nc.dram_tensor("out", [TOK, D], F16, kind="ExternalOutput")

    with tile.TileContext(nc) as tc:
        with tc.tile_pool(name="const", bufs=1) as cpool, \
             tc.tile_pool(name="xp", bufs=8) as xpool, \
             tc.tile_pool(name="bt", bufs=1) as bpool, \
             tc.tile_pool(name="qk", bufs=2) as qkpool, \
             tc.tile_pool(name="avp", bufs=2) as avpool, \
             tc.tile_pool(name="pp", bufs=4) as ppool, \
             tc.tile_pool(name="s2", bufs=6) as s2pool, \
             tc.tile_pool(name="rc", bufs=2) as rpool, \
             tc.tile_pool(name="ot", bufs=3) as opool, \
             tc.tile_pool(name="ps", bufs=1, space="PSUM") as ps:

            wq_sb = cpool.tile([128, NDC, DC], F16)
            wk_sb = cpool.tile([128, NDC, DC], F16)
            wv_sb = cpool.tile([128, NDC, DC], F16)
            wo_sb = cpool.tile([128, HP, D], F16)
            ones_sb = cpool.tile([128, 128], F16)
            bq_sb = cpool.tile([HD, HP], F32)
            bk_sb = cpool.tile([HD, HP], F32)

            wq_r = wq.rearrange("(c p) n -> p c n", p=128)
            wk_r = wk.rearrange("(c p) n -> p c n", p=128)
            wv_r = wv.rearrange("(c p) n -> p c n", p=128)
            wo_r = wo.rearrange("(c p) n -> p c n", p=128)
            xt_r = xt.rearrange("(c p) t -> p c t", p=128)

            nc.sync.dma_start(out=bq_sb, in_=bq2[:, :])
            nc.sync.dma_start(out=bk_sb, in_=bk2[:, :])

            xts = {}

            def x_dma(b, sp):
                t0 = b * S + sp * SPAN
                xtl = xpool.tile([128, NDC, SPAN], F16, name=f"x{b}_{sp}",
                                 tag="x")
                xts[(b, sp)] = xtl
                nc.sync.dma_start(out=xtl, in_=xt_r[:, :, t0:t0 + SPAN])

            # --- b=0 startup: wq chunks interleave with the first x span;
            # remaining x spans stream before wk/wv (the Q pass is DMA-paced,
            # the K/V passes run much later) ---
            x00 = xpool.tile([128, NDC, SPAN], F16, name="x0_0", tag="x")
            xts[(0, 0)] = x00
            for i in range(4):
                nc.sync.dma_start(out=wq_sb[:, 4 * i:4 * i + 4, :],
                                  in_=wq_r[:, 4 * i:4 * i + 4, :])
                nc.sync.dma_start(out=x00[:, 4 * i:4 * i + 4, :],
                                  in_=xt_r[:, 4 * i:4 * i + 4, 0:SPAN])
            for sp in range(1, NSPAN):
                x_dma(0, sp)
            for i in range(2):
                nc.sync.dma_start(out=wk_sb[:, 8 * i:8 * i + 8, :],
                                  in_=wk_r[:, 8 * i:8 * i + 8, :])
            for i in range(2):
                nc.sync.dma_start(out=wv_sb[:, 8 * i:8 * i + 8, :],
                                  in_=wv_r[:, 8 * i:8 * i + 8, :])

            def pull(filler):
                if filler is not None:
                    next(filler, None)

            def proj_pass(b, w_sb, b_sb, dst):
                for sp in range(NSPAN):
                    xtl = xts[(b, sp)]
                    for h in range(HP):
                        pps = ps.tile([128, SPAN], F32, name="pps", tag="pj",
                                      bufs=2)
                        for c in range(NDC):
                            nc.tensor.matmul(
                                pps, w_sb[:, c, h * HD:(h + 1) * HD],
                                xtl[:, c, :], start=(c == 0),
                                stop=(c == NDC - 1))
                        nc.scalar.activation(
                            dst[:, h, sp * SPAN:(sp + 1) * SPAN], pps, Ident,
                            bias=b_sb[:, h:h + 1])

            def v_pass(b, v_b):
                for sp in range(NSPAN):
                    xtl = xts[(b, sp)]
                    vps = ps.tile([128, 2 * DC], F32, name="vps", tag="pj",
                                  bufs=2)
                    for tch in range(2):
                        for c in range(NDC):
                            nc.tensor.matmul(
                                vps[:, tch * DC:(tch + 1) * DC],
                                xtl[:, c, tch * 128:(tch + 1) * 128],
                                wv_sb[:, c, :], start=(c == 0),
                                stop=(c == NDC - 1))
                    for tch in range(2):
                        nc.scalar.copy(v_b[:, sp * 2 + tch, :],
                                       vps[:, tch * DC:(tch + 1) * DC])

            def attn_span(qs, h, qt_b, kt_b, v_b, avt_b, filler=None):
                q_sl = qt_b[:, h, qs * QS:(qs + 1) * QS]
                av_ps = ps.tile([HD, QS], F32, name="av_ps", tag="acc",
                                bufs=2)
                p_tiles = []
                t_tiles = []
                dn_ps = None

                def emit_av(kp):
                    pt = p_tiles[kp]
                    for j in range(2):
                        kc = 2 * kp + j
                        nc.tensor.matmul(
                            av_ps, v_b[:, kc, h * HD:(h + 1) * HD],
                            pt[:, j * QS:(j + 1) * QS], start=(kc == 0),
                            stop=(kc == NKC - 1))

                def dn_mm(src, first=False, last=False):
                    nc.tensor.matmul(dn_ps, ones_sb, src[:, 0:QS],
                                     start=first, stop=False)
                    nc.tensor.matmul(dn_ps, ones_sb, src[:, QS:2 * QS],
                                     start=False, stop=last)

                for kp in range(NPAIR):
                    s_ps = ps.tile([128, 2 * QS], F32, name="s_ps", tag="s",
                                   bufs=2)
                    for j in range(2):
                        kc = 2 * kp + j
                        nc.tensor.matmul(
                            s_ps[:, j * QS:(j + 1) * QS],
                            kt_b[:, h, kc * 128:(kc + 1) * 128], q_sl,
                            start=True, stop=True)
                    pt = ppool.tile([128, 2 * QS], F16, name="p_sb", tag="p")
                    nc.scalar.activation(pt, s_ps, Exp, scale=SCALE)
                    p_tiles.append(pt)
                    # pair-sum tree: early pairs on the idle GPSIMD
                    # (latency-tolerant), later levels on the DVE; the last
                    # exp pair feeds the ones-matmuls directly so the span
                    # tail never waits an add
                    if kp in (1, 3, 5):
                        tt = s2pool.tile([128, 2 * QS], F16, name="t_sb",
                                         tag="s2")
                        eng = nc.gpsimd if kp < 5 else nc.vector
                        eng.tensor_add(tt, p_tiles[kp - 1], p_tiles[kp])
                        t_tiles.append(tt)
                        if kp == 3:
                            ut = s2pool.tile([128, 2 * QS], F16, name="u_sb",
                                             tag="s2")
                            nc.vector.tensor_add(ut, t_tiles[0], t_tiles[1])
                            t_tiles.append(ut)
                        elif kp == 5:
                            wt = s2pool.tile([128, 2 * QS], F16, name="w_sb",
                                             tag="s2")
                            nc.vector.tensor_add(wt, t_tiles[3], t_tiles[2])
                            t_tiles.append(wt)
                    if kp >= 1:
                        emit_av(kp - 1)
                    if kp >= 2:
                        pull(filler)
                emit_av(NPAIR - 1)
                dn_ps = ps.tile([128, QS], F32, name="dn_ps", tag="acc",
                                bufs=2)
                dn_mm(p_tiles[6], first=True)
                dn_mm(p_tiles[7])
                dn_mm(t_tiles[4], last=True)
                recip = rpool.tile([128, QS], F32, name="recip", tag="rc")
                nc.vector.reciprocal_approx_fast(recip, dn_ps)
                nc.vector.tensor_mul(
                    avt_b[:, h, qs * QS:(qs + 1) * QS], av_ps, recip)
                pull(filler)
                pull(filler)

            def outproj_gen(b, qs, avt_b, split):
                for tloc in range(QS // 128):
                    tch = qs * (QS // 128) + tloc
                    out_sb = opool.tile([128, D], F16, name="out_sb",
                                        tag="ot")
                    for dsp in range(4):
                        ops = ps.tile([128, 512], F32, name="ops", tag="pj",
                                      bufs=2)
                        for h in range(HP):
                            nc.tensor.matmul(
                                ops, avt_b[:, h, tch * 128:(tch + 1) * 128],
                                wo_sb[:, h, dsp * 512:(dsp + 1) * 512],
                                start=(h == 0), stop=(h == HP - 1))
                        if split[dsp] == "v":
                            nc.vector.tensor_copy(
                                out_sb[:, dsp * 512:(dsp + 1) * 512], ops)
                        else:
                            nc.scalar.copy(
                                out_sb[:, dsp * 512:(dsp + 1) * 512], ops)
                        if dsp == 3:
                            nc.sync.dma_start(
                                out=out[b * S + tch * 128:
                                        b * S + (tch + 1) * 128, :],
                                in_=out_sb)
                        yield

            carry = None          # half-consumed outproj of (b-1, qs=3)
            for b in range(B):
                qt_b = qkpool.tile([128, HP, S], F16, name="qt_b", tag="qt")
                kt_b = qkpool.tile([128, HP, S], F16, name="kt_b", tag="kt")
                v_b = bpool.tile([128, NKC, DC], F16, name="v_b", tag="v")
                avt_b = avpool.tile([128, HP, S], F16, name="avt_b",
                                    tag="avt")

                proj_pass(b, wq_sb, bq_sb, qt_b)
                proj_pass(b, wk_sb, bk_sb, kt_b)
                v_pass(b, v_b)

                if b == 0:
                    for i in range(4):
                        nc.sync.dma_start(
                            out=wo_sb[:, :, 512 * i:512 * (i + 1)],
                            in_=wo_r[:, :, 512 * i:512 * (i + 1)])
                    nc.sync.dma_start(out=ones_sb, in_=ones[:, :])

                for qs in range(NQS):
                    if qs == 0:
                        filler = carry       # leftovers (may be exhausted)
                    else:
                        filler = outproj_gen(b, qs - 1, avt_b, "vvvv")
                    attn_span(qs, 0, qt_b, kt_b, v_b, avt_b, filler)
                    if qs == 0 and b + 1 < B:
                        for sp in range(NSPAN):
                            x_dma(b + 1, sp)
                    attn_span(qs, 1, qt_b, kt_b, v_b, avt_b, filler)
                    if filler is not None:
                        for _ in filler:     # drain any leftovers
                            pass
                carry = outproj_gen(b, NQS - 1, avt_b,
                                    "vvvv" if b + 1 < B else "svsv")

            if carry is not None:            # last batch's final span:
                for _ in carry:                  # drain with copies split
                    pass                         # across ACT+DVE (both idle)
    nc.compile()
    _BUILT = nc
    return nc


def _install_trace_hooks():
    import types
    try:
        import antenv.axon_hooks  # noqa: F401
        return True
    except ImportError:
        pass
    try:
        from trn_agent_boot.trn_boot import _ntff_profile_via_ctypes
        hook = _ntff_profile_via_ctypes('/opt/axon/libaxon_pjrt.so')
        if hook is None:
            return False
        m = types.ModuleType('antenv.axon_hooks')
        m.get_axon_ntff_profile_hook = lambda: hook
        sys.modules['antenv.axon_hooks'] = m
        from concourse import bass_utils
        bass_utils.upload_artifacts = lambda tmpdir: "local://" + tmpdir
        return True
    except Exception:
        return False


def kernel(x, wq, bq, wk, bk, wv, bv, wo, bo):
    global LAST_EXEC_NS
    from concourse.bass_utils import run_bass_kernel_spmd

    x = np.asarray(x, dtype=np.float32)
    wq = np.asarray(wq, dtype=np.float32)
    bq = np.asarray(bq, dtype=np.float32)
    wk = np.asarray(wk, dtype=np.float32)
    bk = np.asarray(bk, dtype=np.float32)
    wv = np.asarray(wv, dtype=np.float32)
    bv = np.asarray(bv, dtype=np.float32)
    wo = np.asarray(wo, dtype=np.float32)
    bo = np.asarray(bo, dtype=np.float32)

    xt = np.ascontiguousarray(x.reshape(TOK, D).T).astype(np.float16)
    ones = np.ones((128, 128), dtype=np.float16)
    in_maps = []
    for i in range(NCORES):
        sl = slice(i * DC, (i + 1) * DC)
        in_maps.append({
            "xt": xt,
            "wq": np.ascontiguousarray(wq[:, sl]).astype(np.float16),
            "wk": np.ascontiguousarray(wk[:, sl]).astype(np.float16),
            "wv": np.ascontiguousarray(wv[:, sl]).astype(np.float16),
            "wo": np.ascontiguousarray(wo[sl, :]).astype(np.float16),
            "bq2": np.ascontiguousarray(bq[sl].reshape(HP, HD).T),
            "bk2": np.ascontiguousarray(bk[sl].reshape(HP, HD).T),
            "ones": ones,
        })

    trace = bool(os.environ.get("KERNEL_TRACE"))
    if trace:
        trace = _install_trace_hooks()

    nc = _build()
    res = run_bass_kernel_spmd(nc, in_maps, list(range(NCORES)), trace=trace)
    LAST_EXEC_NS = res.exec_time_ns

    total = np.zeros((TOK, D), dtype=np.float32)
    for r in res.results:
        total += r["out"]
    # V-bias folds into a constant row: softmax rows sum to 1, so
    # attention(V + 1*bv^T) = attention(V) + 1*bv^T, and (bv @ wo) adds to bo.
    total += bo + bv @ wo
    return total.reshape(B, S, D)
